# revision 3
# baseline (speedup 1.0000x reference)
"""KappaGCN layer on 8 NeuronCores (Trainium2, Bass/Tile).

Strategy (row-parallel, matching the sharding hint):
  - Each core c owns output rows [c*1024, (c+1)*1024).
  - Host passes A_hat[rows].T contiguous (and bf16-cast) per core, so the
    PE streams A as the moving operand with zero on-chip transposes.
  - Every core redundantly computes the cheap Mobius matvec prologue
    (XW, gamma) for all 8192 nodes, builds a 66-column right-hand side
    [gamma*XW | gamma-2 | 1], and does ONE accumulated matmul
    A_slice @ Bext giving nom, A@(gamma-2), rowsum(A) in one pass.
  - Epilogue (gyromidpoint + mobius scalar mul + expmap0(relu(logmap0)))
    runs on-device in row layout after a small PE transpose.
  - ACT only ever uses the {Ln, Exp} table set: sqrt(x)=exp(0.5 ln x),
    tanh(z)=1-2/(exp(2z)+1), artanh(x)=0.5 ln((1+x)/(1-x)).
"""

import json
import sys

sys.path.insert(0, "/opt/trn_rl_repo")

import ml_dtypes
import numpy as np

import concourse.bass as bass
import concourse.tile as tile
from concourse import mybir
from concourse.bass_utils import run_bass_kernel_spmd
from concourse.masks import make_identity

N, D = 8192, 64
NCORES = 8
ROWS = N // NCORES          # 1024 rows per core
T = N // 128                # 64 node chunks of 128
TC = ROWS // 128            # 8 output chunks per core
EPS = 1e-7
MIN_NORM = 1e-15
BF16 = mybir.dt.bfloat16
F32 = mybir.dt.float32
AF = mybir.ActivationFunctionType
ALU = mybir.AluOpType
X_AX = mybir.AxisListType.X

A_DT = BF16  # A_hat matmul dtype (bf16 halves the HBM-bound DMA traffic)


def _patch_bir_waits(bir_bytes: bytes, max_waits: int = 1) -> bytes:
    """This walrus build only encodes 1 sem-wait per CTRL instruction.
    Split excess waits onto side-effect-free Drain carriers."""
    m = json.loads(bir_bytes)
    uid = [0]
    for fn in m.get("functions", []):
        for blk in fn.get("blocks", []):
            out = []
            for ins in blk.get("instructions", []):
                sync = ins.get("sync_info")
                waits = (sync or {}).get("on_wait") or []
                if sync is not None and len(waits) > max_waits:
                    head = waits[: len(waits) - max_waits]
                    for ci in range(0, len(head), max_waits):
                        uid[0] += 1
                        carrier = {
                            "name": f"{ins['name']}_wsplit{uid[0]}",
                            "opcode": "Drain",
                            "engine": ins["engine"],
                            "ins": [],
                            "outs": [],
                            "is_reset_sema": False,
                            "sync_info": {
                                "on_wait": head[ci: ci + max_waits],
                                "on_update": [],
                            },
                        }
                        if "debug" in ins:
                            carrier["debug"] = ins["debug"]
                        out.append(carrier)
                    sync["on_wait"] = waits[len(waits) - max_waits:]
                out.append(ins)
            blk["instructions"] = out
    return json.dumps(m).encode()


def _artanh_ln2(nc, pool, x, name):
    """Return tile = ln((1+x)/(1-x)) = 2*artanh(x). x must be pre-clipped."""
    a = pool.tile([128, x.shape[1]], F32, name=f"{name}_a")
    b = pool.tile([128, x.shape[1]], F32, name=f"{name}_b")
    nc.vector.tensor_scalar(a, x, -1.0, 1.0, ALU.mult, ALU.add)      # 1-x
    nc.vector.reciprocal(a, a)
    nc.vector.tensor_scalar_add(b, x, 1.0)                            # 1+x
    nc.vector.tensor_mul(b, b, a)
    nc.scalar.activation(b, b, AF.Ln)
    return b


def _sqrt_clip(nc, pool, x2, floor, name):
    """Return tile = sqrt(max(x2, floor)) via exp(0.5 ln)."""
    s = pool.tile([128, x2.shape[1]], F32, name=f"{name}_s")
    nc.vector.tensor_scalar_max(s, x2, floor)
    nc.scalar.activation(s, s, AF.Ln)
    nc.scalar.activation(s, s, AF.Exp, scale=0.5)
    return s


def _tanh_from_exp(nc, pool, z_ln2, name, pre_mul=None):
    """tanh(0.5 * z_ln2 [* pre_mul]) = 1 - 2/(exp(z)+1) where z = z_ln2[*pre_mul].

    z_ln2 already carries the factor 2 (it is 2*artanh-style), so no scaling
    is needed before Exp."""
    e = pool.tile([128, z_ln2.shape[1]], F32, name=f"{name}_e")
    if pre_mul is not None:
        nc.vector.tensor_mul(e, z_ln2, pre_mul)
        nc.scalar.activation(e, e, AF.Exp)
    else:
        nc.scalar.activation(e, z_ln2, AF.Exp)
    nc.vector.tensor_scalar_add(e, e, 1.0)
    nc.vector.reciprocal(e, e)
    nc.vector.tensor_scalar(e, e, -2.0, 1.0, ALU.mult, ALU.add)       # 1-2/(e+1)
    return e


def _build_program():
    nc = bass.Bass()
    at_d = nc.declare_dram_parameter("AT", [N, ROWS], A_DT, isOutput=False)
    xt2_d = nc.declare_dram_parameter("XT2", [128, N // 2], F32, isOutput=False)
    w_d = nc.declare_dram_parameter("WM", [D, D], F32, isOutput=False)
    o_d = nc.declare_dram_parameter("O", [128, TC * D], F32, isOutput=True)

    with tile.TileContext(nc) as tc:
        with (
            tc.tile_pool(name="const", bufs=1) as const,
            tc.tile_pool(name="apool", bufs=6) as apool,
            tc.tile_pool(name="psbig", bufs=2, space="PSUM") as psbig,
            tc.tile_pool(name="psacc", bufs=1, space="PSUM") as psacc,
        ):
            def ct(shape, dt=F32, name=None):
                return const.tile(shape, dt, name=name)

            xt2 = ct([128, N // 2], name="xt2")
            nc.sync.dma_start(xt2, xt2_d[:])
            # W and ones duplicated in both partition halves so rhs base
            # partition matches lhsT chunks at base 0 and base 64.
            w_sb = ct([128, D], name="w_sb")
            nc.sync.dma_start(w_sb[0:64, :], w_d[:])
            nc.sync.dma_start(w_sb[64:128, :], w_d[:])
            ones128 = ct([128, 1], name="ones128")
            nc.vector.memset(ones128, 1.0)
            ident = ct([128, 128], name="ident")
            make_identity(nc, ident)

            def xchunk(buf, t):
                if t < T // 2:
                    return buf[0:64, t * 128:(t + 1) * 128], 0
                return (buf[64:128,
                            (t - T // 2) * 128:(t - T // 2 + 1) * 128], 64)

            # ---- squared X (feeds row norms) ----
            xsq = ct([128, N // 2], name="xsq")
            nc.vector.tensor_mul(xsq, xt2, xt2)

            # ---- nx2[p, t] = ||X_row||^2 via PE (xsq chunk @ ones) ----
            nx2 = ct([128, T], name="nx2")
            for g in range(8):
                ps = psbig.tile([128, 512], F32, name="big")
                for j in range(8):
                    t = g * 8 + j
                    lhsT, bp = xchunk(xsq, t)
                    nc.tensor.matmul(ps[:, j:j + 1], lhsT,
                                     ones128[bp:bp + 64, :],
                                     start=True, stop=True)
                nc.scalar.copy(nx2[:, g * 8:(g + 1) * 8], ps[:, 0:8])

            # ---- mx = X @ W in row layout ----
            mx = ct([128, T, D], name="mx")
            for g in range(8):
                ps = psbig.tile([128, 512], F32, name="big")
                for j in range(8):
                    t = g * 8 + j
                    lhsT, bp = xchunk(xt2, t)
                    nc.tensor.matmul(ps[:, j * 64:(j + 1) * 64],
                                     lhsT, w_sb[bp:bp + 64, :],
                                     start=True, stop=True)
                nc.scalar.copy(mx[:, g * 8:(g + 1) * 8, :], ps)

            # ---- nmx2 = row norms^2 of mx ----
            mxsq = ct([128, T, D], name="mxsq")
            nc.vector.tensor_mul(mxsq, mx, mx)
            nmx2 = ct([128, T], name="nmx2")
            nc.vector.reduce_sum(nmx2, mxsq, axis=X_AX)

            # ---- Mobius matvec scalars ----
            nx = _sqrt_clip(nc, const, nx2, MIN_NORM, "nx")
            nmx = _sqrt_clip(nc, const, nmx2, MIN_NORM, "nmx")
            nxc = ct([128, T], name="nxc")
            nc.vector.tensor_scalar_min(nxc, nx, 1.0 - EPS)
            lnr1 = _artanh_ln2(nc, const, nxc, "at1")                 # 2*artanh(nx)
            q = ct([128, T], name="q")
            nc.vector.reciprocal(q, nx)
            nc.vector.tensor_mul(q, nmx, q)                           # nmx/nx
            th = _tanh_from_exp(nc, const, lnr1, "th", pre_mul=q)     # tanh(nmx/nx*artanh(nx))
            rnmx = ct([128, T], name="rnmx")
            nc.vector.reciprocal(rnmx, nmx)
            scal = ct([128, T], name="scal")
            nc.vector.tensor_mul(scal, th, rnmx)                      # |XW| coef: XW = scal*mx
            # gamma = 2 / max(1 - th^2, EPS)   (since ||XW|| = th)
            om = ct([128, T], name="om")
            nc.vector.tensor_mul(om, th, th)
            nc.vector.tensor_scalar(om, om, -1.0, 1.0, ALU.mult, ALU.add)
            nc.vector.tensor_scalar_max(om, om, EPS)
            gamma = ct([128, T], name="gamma")
            nc.vector.reciprocal(gamma, om)
            nc.vector.tensor_scalar_mul(gamma, gamma, 2.0)
            coef = ct([128, T], name="coef")
            nc.vector.tensor_mul(coef, gamma, scal)                   # gamma*scal
            gm2 = ct([128, T], name="gm2")
            nc.vector.tensor_scalar_add(gm2, gamma, -2.0)             # gamma-2 (tiny, bf16-safe)

            # ---- Bext [128, T, 66] = [gamma*XW | gamma-2 | 1] ----
            bext = ct([128, T, 66], A_DT, name="bext")
            nc.vector.memset(bext[:, :, 65:66], 1.0)
            nc.vector.tensor_copy(bext[:, :, 64:65], gm2[:, :, None])
            nc.vector.tensor_tensor(
                bext[:, :, 0:64], mx,
                coef[:, :, None].to_broadcast(mx.shape), ALU.mult)

            # ---- big matmul: out.T = Bext_k.T @ A.T, accumulated over k ----
            ps_lo = psacc.tile([66, 512], F32, name="ps_lo")
            ps_hi = psacc.tile([66, 512], F32, name="ps_hi")
            atr = at_d[:].rearrange("(kt p) m -> p kt m", p=128)
            for g in range(8):
                at = apool.tile([128, 8, ROWS], A_DT, name="at")
                nc.sync.dma_start(at, atr[:, g * 8:(g + 1) * 8, :])
                for j in range(8):
                    kt = g * 8 + j
                    lhsT = bext[:, kt, :]
                    nc.tensor.matmul(ps_lo, lhsT, at[:, j, 0:512],
                                     start=(kt == 0), stop=(kt == T - 1))
                    nc.tensor.matmul(ps_hi, lhsT, at[:, j, 512:1024],
                                     start=(kt == 0), stop=(kt == T - 1))

            outT = ct([66, ROWS], name="outT")
            nc.vector.tensor_copy(outT[:, 0:512], ps_lo)
            nc.vector.tensor_copy(outT[:, 512:1024], ps_hi)

            # ---- transpose back to row layout [128, TC, 66] ----
            og = ct([128, TC, 66], name="og")
            for c in range(TC):
                pst = psbig.tile([128, 512], F32, name="big")
                nc.tensor.transpose(pst[:, 0:66],
                                    outT[:, c * 128:(c + 1) * 128],
                                    ident[0:66, 0:66])
                nc.vector.tensor_copy(og[:, c, :], pst[:, 0:66])

            # ---- epilogue (row layout; per-row scalars are [128, TC]) ----
            def e8(name):
                return const.tile([128, TC], F32, name=name)

            nom = og[:, :, 0:64]
            den = e8("den")
            nc.vector.tensor_add(den, og[:, :, 64], og[:, :, 65])     # A@(g-2) + r
            nc.vector.tensor_scalar_max(den, den, 1e-10)
            rden = e8("rden")
            nc.vector.reciprocal(rden, den)
            tm = ct([128, TC, D], name="tm")                          # two_mean
            nc.vector.tensor_tensor(tm, nom,
                                    rden[:, :, None].to_broadcast(tm.shape),
                                    ALU.mult)
            tmsq = ct([128, TC, D], name="tmsq")
            nc.vector.tensor_mul(tmsq, tm, tm)
            sq = e8("sq")
            nc.vector.reduce_sum(sq, tmsq, axis=X_AX)
            om1 = e8("om1")
            nc.vector.tensor_scalar(om1, sq, -1.0, 1.0, ALU.mult, ALU.add)
            s1 = _sqrt_clip(nc, const, om1, 1e-30, "s1")              # sqrt(max(1-sq,0))
            nc.vector.tensor_scalar_add(s1, s1, 1.0)
            nc.vector.reciprocal(s1, s1)
            mid = ct([128, TC, D], name="mid")
            nc.vector.tensor_tensor(mid, tm,
                                    s1[:, :, None].to_broadcast(mid.shape),
                                    ALU.mult)
            # mobius_scalar_mul(r, mid)
            midsq = ct([128, TC, D], name="midsq")
            nc.vector.tensor_mul(midsq, mid, mid)
            m2 = e8("m2")
            nc.vector.reduce_sum(m2, midsq, axis=X_AX)
            nm = _sqrt_clip(nc, const, m2, MIN_NORM, "nm")
            nmcl = e8("nmcl")
            nc.vector.tensor_scalar_min(nmcl, nm, 1.0 - EPS)
            lnr2 = _artanh_ln2(nc, const, nmcl, "at2")
            th2 = _tanh_from_exp(nc, const, lnr2, "th2",
                                 pre_mul=og[:, :, 65])                # tanh(r*artanh(nm))
            c1 = e8("c1")
            nc.vector.reciprocal(c1, nm)
            nc.vector.tensor_mul(c1, th2, c1)
            axw = ct([128, TC, D], name="axw")
            nc.vector.tensor_tensor(axw, mid,
                                    c1[:, :, None].to_broadcast(axw.shape),
                                    ALU.mult)
            # logmap0 + relu + expmap0
            axwsq = ct([128, TC, D], name="axwsq")
            nc.vector.tensor_mul(axwsq, axw, axw)
            a2 = e8("a2")
            nc.vector.reduce_sum(a2, axwsq, axis=X_AX)
            n2 = _sqrt_clip(nc, const, a2, MIN_NORM, "n2")
            n2c = e8("n2c")
            nc.vector.tensor_scalar_min(n2c, n2, 1.0 - EPS)
            lnr3 = _artanh_ln2(nc, const, n2c, "at3")
            uc = e8("uc")
            nc.vector.reciprocal(uc, n2)
            nc.vector.tensor_mul(uc, lnr3, uc)
            nc.vector.tensor_scalar_mul(uc, uc, 0.5)                  # artanh(n2)/n2
            vr = ct([128, TC, D], name="vr")
            nc.vector.tensor_scalar_max(vr, axw, 0.0)                 # relu(AXW)
            wv = ct([128, TC, D], name="wv")
            nc.vector.tensor_tensor(wv, vr,
                                    uc[:, :, None].to_broadcast(wv.shape),
                                    ALU.mult)                          # relu(logmap0)
            wvsq = ct([128, TC, D], name="wvsq")
            nc.vector.tensor_mul(wvsq, wv, wv)
            w2 = e8("w2")
            nc.vector.reduce_sum(w2, wvsq, axis=X_AX)
            n3 = _sqrt_clip(nc, const, w2, MIN_NORM, "n3")
            # tanh(n3) = 1 - 2/(exp(2*n3)+1)
            e3 = e8("e3")
            nc.scalar.activation(e3, n3, AF.Exp, scale=2.0)
            nc.vector.tensor_scalar_add(e3, e3, 1.0)
            nc.vector.reciprocal(e3, e3)
            nc.vector.tensor_scalar(e3, e3, -2.0, 1.0, ALU.mult, ALU.add)
            c3 = e8("c3")
            nc.vector.reciprocal(c3, n3)
            nc.vector.tensor_mul(c3, e3, c3)
            oo = ct([128, TC, D], name="oo")
            nc.vector.tensor_tensor(oo, wv,
                                    c3[:, :, None].to_broadcast(oo.shape),
                                    ALU.mult)
            nc.sync.dma_start(o_d[:].rearrange("p (tc d) -> p tc d", tc=TC), oo)

    orig = bass.Bass.to_json_bytes
    nc.to_json_bytes = lambda: _patch_bir_waits(orig(nc))
    return nc


_NC_CACHE = None


def kernel(X, A_hat, W):
    global _NC_CACHE
    if _NC_CACHE is None:
        _NC_CACHE = _build_program()
    nc = _NC_CACHE

    X = np.asarray(X, np.float32)
    A_hat = np.asarray(A_hat, np.float32)
    W = np.asarray(W, np.float32)

    xt = np.ascontiguousarray(X.T)                       # [64, 8192]
    xt2 = np.concatenate([xt[:, :N // 2], xt[:, N // 2:]], axis=0)  # [128, 4096]
    np_adt = mybir.dt.np(A_DT)
    in_maps = []
    for c in range(NCORES):
        at = np.ascontiguousarray(A_hat[c * ROWS:(c + 1) * ROWS, :].T)
        in_maps.append({
            "AT": at.astype(np_adt),
            "XT2": xt2,
            "WM": W,
        })
    res = run_bass_kernel_spmd(nc, in_maps, list(range(NCORES)))
    out = np.empty((N, D), np.float32)
    for c in range(NCORES):
        o = np.asarray(res.results[c]["O"], np.float32)
        out[c * ROWS:(c + 1) * ROWS] = (
            o.reshape(128, TC, D).transpose(1, 0, 2).reshape(ROWS, D))
    return out
